# revision 65
# baseline (speedup 1.0000x reference)
"""Trainium2 Bass kernel for the attention-based encoder.

Computation (per batch b):
    a      = P @ y[b]                                  # [D]
    logits = x[b] @ a                                  # [M]
    p_un   = exp(logits - 16); Z = sum(p_un)
    W[t]   = p_un[t-1] + p_un[t] + p_un[t+1] + p_un[t+2]  (zero-padded), W[M-1] = 0
    enc[b] = (W @ x[b]) / (Q * Z)                      # [D]

which is algebraically identical to the reference (cumsum sliding window +
bilinear softmax attention), with the smoothing window folded onto the softmax
weights instead of the embeddings so x[b] is only needed in natural layout.

Sharding: data-parallel over batch, 4 batches per core on 8 cores. P is
replicated (passed pre-transposed so the contraction dim lands on SBUF
partitions without on-chip transposes).

Precision: x, P and y are cast to bf16 on the host (halves HBM traffic, the
dominant cost, and runs the PE/DVE paths at 2x fp32 rate). All reductions
(logit accumulate, softmax Z, PSUM matmul accumulate) stay fp32. Measured
end-to-end rel err vs the fp32 reference is ~5e-3.
"""

import numpy as np
import ml_dtypes

import concourse.bass as bass
import concourse.mybir as mybir
from concourse.tile import TileContext
from concourse.bass_utils import run_bass_kernel_spmd

# ---------------------------------------------------------------------------
# This container's walrus supports only ONE sync wait per instruction ("Too
# many sync wait commands" at codegen otherwise), while Tile freely attaches
# several.  Post-pass: hoist excess waits onto injected same-engine NoOps
# placed immediately before the over-subscribed instruction.
# ---------------------------------------------------------------------------

_MAX_WAITS = 1


def split_sync_waits(nc: bass.Bass) -> None:
    uid = 0
    for fn in nc.m.functions:
        for blk in fn.blocks:
            new_insts = []
            for inst in blk.instructions:
                si = inst.sync_info
                waits = list(si.on_wait) if si and si.on_wait else []
                if len(waits) > _MAX_WAITS:
                    for w in waits[:-_MAX_WAITS]:
                        uid += 1
                        ev = mybir.InstEventSemaphore(
                            name=f"{inst.name}_hw{uid}",
                            opcode="EventSemaphore",
                            ins=[],
                            outs=[],
                            sync_info=mybir.SyncInfo(on_wait=[w], on_update=[]),
                        )
                        ev.engine = inst.engine
                        new_insts.append(ev)
                    si.on_wait = waits[-_MAX_WAITS:]
                new_insts.append(inst)
            blk.instructions[:] = new_insts

# ---------------------------------------------------------------------------

B, M, D, CD = 32, 2048, 1024, 5120
Q = 2
NCORES = 8
BPC = B // NCORES          # batches per core
NT = M // 128              # m-tiles per batch
KT = CD // 128             # k-tiles of the P contraction
KTC = KT // NCORES         # k-tiles per core (phase A k-sharded)
F32 = mybir.dt.float32
BF16 = mybir.dt.bfloat16
ALU = mybir.AluOpType
AFT = mybir.ActivationFunctionType


def build_nc(reps: int = 1, n_batches: int = BPC, pipeline: bool = True,
             x_bufs: int = 3, x_chunk: int = 8, wopt: bool = False,
             scale_split: bool = False, dual_q: bool = False,
             defer: bool = False, cc_bf16: bool = False,
             pt_bufs: int = 2, fold_bufs: int = 3,
             out_q_scalar: bool = True) -> bass.Bass:
    nc = bass.Bass()
    xs = nc.declare_dram_parameter("xs", [BPC, M, D], BF16, isOutput=False)
    # per-core k-slice of P^T: [KTC*128, D]
    pt = nc.declare_dram_parameter("pt", [KTC * 128, D], BF16, isOutput=False)
    # y for ALL batches over this core's k-slice: [128, KTC, B]
    ys = nc.declare_dram_parameter("ys", [128, KTC, B], BF16, isOutput=False)
    enc = nc.declare_dram_parameter("enc", [BPC, D], F32, isOutput=True)

    with TileContext(nc) as tc:
        with (
            tc.tile_pool(name="const", bufs=1) as const_pool,
            tc.tile_pool(name="ptp", bufs=pt_bufs) as pt_pool,
            tc.tile_pool(name="xp", bufs=x_bufs) as x_pool,
            tc.tile_pool(name="arep", bufs=2) as arep_pool,
            tc.tile_pool(name="small", bufs=1) as small_pool,
            tc.tile_pool(name="tiny", bufs=2) as tiny_pool,
            tc.tile_pool(name="scr", bufs=3) as scr_pool,
            tc.tile_pool(name="fold", bufs=fold_bufs) as fold_pool,
            tc.tile_pool(name="ps", bufs=1, space="PSUM") as psum_pool,
            tc.tile_pool(name="psw", bufs=1, space="PSUM") as psum_w_pool,
            tc.tile_pool(name="pse", bufs=1, space="PSUM") as psum_e_pool,
            tc.tile_pool(name="psb", bufs=1, space="PSUM") as psum_b_pool,
            tc.tile_pool(name="dram", bufs=2, space="DRAM") as dram_pool,
        ):
            ones_sb = const_pool.tile([1, 128], BF16)
            nc.vector.memset(ones_sb[:], 1.0)
            ones_col = const_pool.tile([128, 1], F32)
            nc.vector.memset(ones_col[:], 1.0)
            nshift = const_pool.tile([128, 1], F32)
            nc.vector.memset(nshift[:], -16.0)
            ys_sb = const_pool.tile([128, KTC, B], BF16)
            # ys/a_row ride the Activation hwdge queue so the SP queue's
            # xb transfers are not blocked behind the phase-A dependency
            nc.scalar.dma_start(out=ys_sb[:], in_=ys[:])

            # banded matrices for W = S4 @ p (4-tap sliding-window sum done
            # as matmuls in partition space): S4[j, m] = 1/Q iff j-m in
            # {-1, 0, 1, 2}; corner matrices carry the inter-tile halo. The
            # 1/Q entry folds the final "/Q" divide into the band weights.
            FILL = 1.0 / Q
            s4 = const_pool.tile([128, 128], F32)
            nc.gpsimd.memset(s4[:], 0.0)
            for base in (1, 0, -1, -2):
                nc.gpsimd.affine_select(
                    out=s4[:], in_=s4[:], compare_op=ALU.not_equal, fill=FILL,
                    base=base, pattern=[[-1, 128]], channel_multiplier=1,
                )
            sprev = const_pool.tile([128, 128], F32)
            nc.gpsimd.memset(sprev[:], 0.0)
            nc.gpsimd.affine_select(
                out=sprev[:], in_=sprev[:], compare_op=ALU.not_equal, fill=FILL,
                base=-127, pattern=[[-1, 128]], channel_multiplier=1,
            )
            snext = const_pool.tile([128, 128], F32)
            nc.gpsimd.memset(snext[:], 0.0)
            for base in (126, 127):
                nc.gpsimd.affine_select(
                    out=snext[:], in_=snext[:], compare_op=ALU.not_equal,
                    fill=FILL, base=base, pattern=[[-1, 128]],
                    channel_multiplier=1,
                )
            zrow = const_pool.tile([1, 128], F32)
            nc.vector.memset(zrow[:], 0.0)

            def phase_a_pre():
                # ---- Phase A (k-sharded): each core contracts its 1/8
                # k-slice of P^T against y for ALL batches, then a
                # ReduceScatter sums the partials and hands each core the
                # rows of its own 4 batches. Split into pre (up to the
                # collective trigger) and post (everything that waits on the
                # collective) so phase A for rep i+1 can be issued around
                # rep i's phase B without stalling the in-order engines. ----
                pa0 = psum_pool.tile([B, 512], F32, tag="pa0")
                pa1 = psum_pool.tile([B, 512], F32, tag="pa1")
                ptt = pt_pool.tile([128, KTC, D], BF16, tag="ptt")
                nc.sync.dma_start(
                    out=ptt[:],
                    in_=pt[:, :].rearrange("(u p) d -> p u d", p=128),
                )
                for t in range(KTC):
                    for dh, pa in enumerate((pa0, pa1)):
                        nc.tensor.matmul(
                            pa[:],
                            lhsT=ys_sb[:, t, :],
                            rhs=ptt[:, t, dh * 512:(dh + 1) * 512],
                            start=(t == 0),
                            stop=(t == KTC - 1),
                        )
                ccdt = BF16 if cc_bf16 else F32
                aT32f = small_pool.tile([B, D], ccdt, tag="aT32f")
                nc.scalar.copy(out=aT32f[:, 0:512], in_=pa0[:])
                nc.scalar.copy(out=aT32f[:, 512:1024], in_=pa1[:])
                cc_in = dram_pool.tile([B, D], ccdt, tag="cc_in")
                cc_out = dram_pool.tile([BPC, D], ccdt, tag="cc_out")
                nc.gpsimd.dma_start(out=cc_in[:], in_=aT32f[:])
                nc.gpsimd.collective_compute(
                    "ReduceScatter",
                    mybir.AluOpType.add,
                    replica_groups=[list(range(NCORES))],
                    ins=[cc_in.opt()],
                    outs=[cc_out.opt()],
                )
                return cc_out

            arep_gen = 0

            def phase_a_post(cc_out):
                if cc_bf16:
                    # the scattered rows are already bf16: land them directly
                    aT16 = small_pool.tile([BPC, D], BF16, tag="aT")
                    nc.sync.dma_start(out=aT16[:], in_=cc_out[:])
                else:
                    aT32m = small_pool.tile([BPC, D], F32, tag="aT32m")
                    nc.sync.dma_start(out=aT32m[:], in_=cc_out[:])
                    aT16 = small_pool.tile([BPC, D], BF16, tag="aT")
                    nc.scalar.copy(out=aT16[:], in_=aT32m[:])

                # replicate a[b] across all 128 partitions (ones ⊗ a-row on
                # PE; the partition_broadcast ISA op does not codegen on this
                # walrus), then duplicate along the free dim on DVE so the
                # phase-B pair-multiplies see [128, 2, D].
                nonlocal arep_gen
                arep_gen += 1
                a_rep = [
                    arep_pool.tile([128, 2, D], BF16, tag=f"a_rep{b}",
                                   name=f"a_rep{b}_g{arep_gen}")
                    for b in range(BPC)
                ]
                for b in range(BPC):
                    a_row = small_pool.tile([1, D], BF16, tag="a_row")
                    nc.scalar.dma_start(out=a_row[:], in_=aT16[b:b + 1, :])
                    for dh in range(2):
                        pr = psum_b_pool.tile([128, 512], F32, tag="pr")
                        nc.tensor.matmul(
                            pr[:],
                            lhsT=ones_sb[:],
                            rhs=a_row[:, dh * 512:(dh + 1) * 512],
                            start=True,
                            stop=True,
                        )
                        nc.scalar.copy(
                            out=a_rep[b][:, 0, dh * 512:(dh + 1) * 512],
                            in_=pr[:],
                        )
                    nc.vector.tensor_copy(a_rep[b][:, 1, :], a_rep[b][:, 0, :])
                return a_rep

            # Deferred batch tails (1/Z + final scale + output DMA): the
            # scale depends on the enc matmul chain (~14us of PE), and ACT
            # is in-order — emitting it inline stalls ACT between exp(b)
            # and the next batch's accumulates. Deferring it into the next
            # batch (reciprocal at batch start on DVE, scale right after
            # exp on ACT) makes every read hit long-completed data.
            pending_tail = []

            def emit_tail_rz(t):
                rz = tiny_pool.tile([1, 1], F32, tag="rz")
                nc.vector.reciprocal(rz[:], t["z_ps"][:])
                t["rz"] = rz

            def emit_tail_scale(t):
                enc_sb = tiny_pool.tile([1, D], F32, tag="enc_sb")
                nc.scalar.activation(
                    out=enc_sb[:, 0:512], in_=t["pe0"][:], func=AFT.Copy,
                    scale=t["rz"][:],
                )
                nc.scalar.activation(
                    out=enc_sb[:, 512:1024], in_=t["pe1"][:], func=AFT.Copy,
                    scale=t["rz"][:],
                )
                nc.sync.dma_start(out=enc[t["b"], :], in_=enc_sb[0:1, :])

            def phase_b(a_rep):
                for b in range(n_batches):
                    if defer and pending_tail:
                        emit_tail_rz(pending_tail[0])
                    xb = x_pool.tile([128, NT, D], BF16, tag="xb")
                    ch = x_chunk
                    for t in range(NT // ch):
                        eng = (nc.scalar if dual_q and t % 2 else nc.sync)
                        eng.dma_start(
                            out=xb[:, ch * t:ch * t + ch, :],
                            in_=xs[b, t * 128 * ch:(t + 1) * 128 * ch, :]
                            .rearrange("(u p) d -> p u d", p=128),
                        )

                    # logits[m] = x[m, :] . a — DVE multiply over two m-tiles
                    # per op, then tree-folds spread across DVE/GpSimd and a
                    # ScalarE Copy-with-accumulate per tile. (The fused
                    # TENSOR_TENSOR_REDUCE / AFFINE_MUL_REDUCE DVE ops are
                    # rejected by this walrus build: "ISA wrong length".)
                    # Pairs 0..4: DVE fold to 512, GpSimd fold to 256, ACT
                    # accumulates 256. Pairs 5..7: GpSimd fold to 512, ACT
                    # accumulates 512. Balances DVE/ACT/Pool near the per-
                    # batch DMA floor.
                    logits_sb = tiny_pool.tile([128, NT], F32, tag="logits")
                    for j in range(NT // 2):
                        scr2 = scr_pool.tile([128, 2, D], BF16, tag="scr2")
                        nc.vector.tensor_mul(
                            scr2[:], xb[:, 2 * j:2 * j + 2, :], a_rep[b][:]
                        )
                        if j < 6:
                            # DVE fold to 512, GpSimd fold to 256, ACT-256
                            half = fold_pool.tile([128, 2, 512], BF16,
                                                  tag="half")
                            nc.vector.tensor_add(
                                half[:], scr2[:, :, 0:512],
                                scr2[:, :, 512:1024]
                            )
                            red = fold_pool.tile([128, 2, 256], BF16, tag="q")
                            nc.gpsimd.tensor_add(
                                red[:], half[:, :, 0:256], half[:, :, 256:512]
                            )
                        else:
                            # GpSimd fold to 512, ACT-512
                            red = fold_pool.tile([128, 2, 512], BF16,
                                                 tag="half")
                            nc.gpsimd.tensor_add(
                                red[:], scr2[:, :, 0:512], scr2[:, :, 512:1024]
                            )
                        for u in range(2):
                            t = 2 * j + u
                            nc.scalar.activation(
                                out=red[:, u, :],
                                in_=red[:, u, :],
                                func=AFT.Copy,
                                accum_out=logits_sb[:, t:t + 1],
                            )

                    # softmax without the row gather: a FIXED shift replaces
                    # the max (it cancels exactly in enc = sum(W x)/(2Z)),
                    # so exp runs in [128, NT] partition space on ACT.
                    p_sb = tiny_pool.tile([128, NT], F32, tag="p_sb")
                    zcol = tiny_pool.tile([128, 1], F32, tag="zcol")
                    nc.scalar.activation(
                        out=p_sb[:],
                        in_=logits_sb[:],
                        func=AFT.Exp,
                        bias=nshift[:],
                        scale=1.0,
                        accum_out=zcol[:],
                    )

                    # Deferred tail of the previous batch: its enc PSUM and
                    # Z are long done, so these ACT ops issue stall-free.
                    if defer and pending_tail:
                        emit_tail_scale(pending_tail.pop(0))

                    # Z = sum over partitions of zcol (ones-column matmul),
                    # then 1/Z on DVE straight from PSUM (the /Q is folded
                    # into the band fills).
                    z_ps = psum_w_pool.tile([1, 1], F32, tag="z_ps")
                    nc.tensor.matmul(z_ps[:], lhsT=zcol[:], rhs=ones_col[:],
                                     start=True, stop=True)
                    if not defer:
                        rz = tiny_pool.tile([1, 1], F32, tag="rz")
                        nc.vector.reciprocal(rz[:], z_ps[:])

                    # W[m] = p[m-1]+p[m]+p[m+1]+p[m+2] via banded matmuls;
                    # the inter-tile halo terms act on p shifted by one tile
                    # column (zero-padded at the ends).
                    w_ps = psum_w_pool.tile([128, NT], F32, tag="w_ps")
                    if wopt:
                        # column-shifted PSUM subranges instead of shifted
                        # copies of p; a zero matmul closes the accumulation
                        # group over the full range (stop is sim-only).
                        nc.tensor.matmul(w_ps[:], lhsT=s4[:], rhs=p_sb[:],
                                         start=True, stop=False)
                        nc.tensor.matmul(w_ps[:, 1:NT], lhsT=sprev[:],
                                         rhs=p_sb[:, 0:NT - 1],
                                         start=False, stop=False)
                        nc.tensor.matmul(w_ps[:, 0:NT - 1], lhsT=snext[:],
                                         rhs=p_sb[:, 1:NT],
                                         start=False, stop=False)
                        nc.tensor.matmul(w_ps[:], lhsT=zrow[:],
                                         rhs=p_sb[0:1, :],
                                         start=False, stop=True)
                    else:
                        p_prev = tiny_pool.tile([128, NT], F32, tag="p_prev")
                        nc.gpsimd.memset(p_prev[:, 0:1], 0.0)
                        nc.gpsimd.tensor_copy(p_prev[:, 1:NT],
                                              p_sb[:, 0:NT - 1])
                        p_next = tiny_pool.tile([128, NT], F32, tag="p_next")
                        nc.gpsimd.memset(p_next[:, NT - 1:NT], 0.0)
                        nc.gpsimd.tensor_copy(p_next[:, 0:NT - 1],
                                              p_sb[:, 1:NT])
                        nc.tensor.matmul(w_ps[:], lhsT=s4[:], rhs=p_sb[:],
                                         start=True, stop=False)
                        nc.tensor.matmul(w_ps[:], lhsT=sprev[:], rhs=p_prev[:],
                                         start=False, stop=False)
                        nc.tensor.matmul(w_ps[:], lhsT=snext[:], rhs=p_next[:],
                                         start=False, stop=True)
                    w_pm = tiny_pool.tile([128, NT], BF16, tag="w_pm")
                    if defer:
                        # adjacent to exp on ACT: no cross-engine stall
                        nc.scalar.copy(out=w_pm[:], in_=w_ps[:])
                    else:
                        nc.vector.tensor_copy(w_pm[:], w_ps[:])

                    # enc_un[d] = sum_m W[m] x[m, d]   (PE, W as 1-col
                    # weights, 1024-wide rhs into a 2-bank PSUM tile).
                    # The last tile contracts over 127 partitions only: this
                    # drops m = M-1, enforcing W[M-1] = 0 (exclusive slice end
                    # in the reference) without touching w_pm.
                    pe0 = psum_e_pool.tile([1, 512], F32, tag="pe0")
                    pe1 = psum_e_pool.tile([1, 512], F32, tag="pe1")
                    for t in range(NT):
                        pp = 127 if t == NT - 1 else 128
                        for dh, pe in enumerate((pe0, pe1)):
                            nc.tensor.matmul(
                                pe[:],
                                lhsT=w_pm[0:pp, t:t + 1],
                                rhs=xb[0:pp, t, dh * 512:(dh + 1) * 512],
                                start=(t == 0),
                                stop=(t == NT - 1),
                            )

                    # enc[b] = enc_un / Z  (the /Q went into the band fills)
                    if defer:
                        pending_tail.append(
                            {"pe0": pe0, "pe1": pe1, "z_ps": z_ps, "b": b}
                        )
                        continue
                    enc_sb = small_pool.tile([1, D], F32, tag="enc_sb")
                    nc.scalar.activation(
                        out=enc_sb[:, 0:512], in_=pe0[:], func=AFT.Copy,
                        scale=rz[:],
                    )
                    if scale_split:
                        nc.vector.tensor_scalar_mul(
                            enc_sb[:, 512:1024], pe1[:], rz[:]
                        )
                    else:
                        nc.scalar.activation(
                            out=enc_sb[:, 512:1024], in_=pe1[:], func=AFT.Copy,
                            scale=rz[:],
                        )
                    oeng = nc.scalar if out_q_scalar else nc.sync
                    oeng.dma_start(out=enc[b, :], in_=enc_sb[0:1, :])

            # Python-unrolled reps (collective_compute does not codegen
            # inside a For_i hardware loop on this walrus build), software-
            # pipelined: rep i's phase B runs while rep i+1's phase A
            # (P-slice matmul + ReduceScatter) is in flight; the collective-
            # dependent tail of phase A is emitted after phase B so no
            # in-order engine stalls waiting on the collective.
            if pipeline:
                a_rep = phase_a_post(phase_a_pre())
                for i in range(reps):
                    cc_next = phase_a_pre() if i + 1 < reps else None
                    phase_b(a_rep)
                    if cc_next is not None:
                        a_rep = phase_a_post(cc_next)
            else:
                for i in range(reps):
                    phase_b(phase_a_post(phase_a_pre()))
            while pending_tail:
                t = pending_tail.pop(0)
                emit_tail_rz(t)
                emit_tail_scale(t)

    return nc


def _shard_inputs(embeds_x, embeds_y, P):
    """Build the 8 per-core input maps (host-side resharding + bf16 cast).

    x is data-parallel over batch (4 per core). Phase A is k-sharded: core c
    gets P^T rows [c*640, (c+1)*640) and y[:, same k-slice] for ALL batches;
    the kernel ReduceScatters the partial a's.
    """
    bf16 = ml_dtypes.bfloat16
    x = np.asarray(embeds_x, dtype=np.float32).astype(bf16)
    y = np.asarray(embeds_y, dtype=np.float32)[:, :, 0].astype(bf16)  # [B, CD]
    pt = np.ascontiguousarray(
        np.asarray(P, dtype=np.float32).T.astype(bf16))      # [CD, D]
    kc = KTC * 128
    in_maps = []
    for c in range(NCORES):
        sl = slice(c * BPC, (c + 1) * BPC)
        ys_c = np.ascontiguousarray(
            y[:, c * kc:(c + 1) * kc].reshape(B, KTC, 128).transpose(2, 1, 0)
        )  # [128, KTC, B]
        in_maps.append({
            "xs": np.ascontiguousarray(x[sl]),
            "pt": np.ascontiguousarray(pt[c * kc:(c + 1) * kc]),
            "ys": ys_c,
        })
    return in_maps


def kernel(embeds_x, embeds_y, P, M):
    assert int(M) == 2048
    nc = build_nc(reps=1)
    split_sync_waits(nc)  # HW-compile only; CoreSim rejects injected NoOps
    in_maps = _shard_inputs(embeds_x, embeds_y, P)
    res = run_bass_kernel_spmd(nc, in_maps, list(range(NCORES)))
    out = np.concatenate([res.results[c]["enc"] for c in range(NCORES)], axis=0)
    return out.astype(np.float32)


# revision 67
# speedup vs baseline: 1.1043x; 1.1043x over previous
"""Trainium2 Bass kernel for the attention-based encoder.

Computation (per batch b):
    a      = P @ y[b]                                  # [D]
    logits = x[b] @ a                                  # [M]
    p_un   = exp(logits - 16); Z = sum(p_un)
    W[t]   = p_un[t-1] + p_un[t] + p_un[t+1] + p_un[t+2]  (zero-padded), W[M-1] = 0
    enc[b] = (W @ x[b]) / (Q * Z)                      # [D]

which is algebraically identical to the reference (cumsum sliding window +
bilinear softmax attention), with the smoothing window folded onto the softmax
weights instead of the embeddings so x[b] is only needed in natural layout.

Sharding: data-parallel over batch, 4 batches per core on 8 cores. P is
replicated (passed pre-transposed so the contraction dim lands on SBUF
partitions without on-chip transposes).

Precision: x, P and y are cast to bf16 on the host (halves HBM traffic, the
dominant cost, and runs the PE/DVE paths at 2x fp32 rate). All reductions
(logit accumulate, softmax Z, PSUM matmul accumulate) stay fp32. Measured
end-to-end rel err vs the fp32 reference is ~5e-3.
"""

import numpy as np
import ml_dtypes

import concourse.bass as bass
import concourse.mybir as mybir
from concourse.tile import TileContext
from concourse.bass_utils import run_bass_kernel_spmd

# ---------------------------------------------------------------------------
# This container's walrus supports only ONE sync wait per instruction ("Too
# many sync wait commands" at codegen otherwise), while Tile freely attaches
# several.  Post-pass: hoist excess waits onto injected same-engine NoOps
# placed immediately before the over-subscribed instruction.
# ---------------------------------------------------------------------------

_MAX_WAITS = 1


def split_sync_waits(nc: bass.Bass) -> None:
    uid = 0
    for fn in nc.m.functions:
        for blk in fn.blocks:
            new_insts = []
            for inst in blk.instructions:
                si = inst.sync_info
                waits = list(si.on_wait) if si and si.on_wait else []
                if len(waits) > _MAX_WAITS:
                    for w in waits[:-_MAX_WAITS]:
                        uid += 1
                        ev = mybir.InstEventSemaphore(
                            name=f"{inst.name}_hw{uid}",
                            opcode="EventSemaphore",
                            ins=[],
                            outs=[],
                            sync_info=mybir.SyncInfo(on_wait=[w], on_update=[]),
                        )
                        ev.engine = inst.engine
                        new_insts.append(ev)
                    si.on_wait = waits[-_MAX_WAITS:]
                new_insts.append(inst)
            blk.instructions[:] = new_insts

# ---------------------------------------------------------------------------

B, M, D, CD = 32, 2048, 1024, 5120
Q = 2
NCORES = 8
BPC = B // NCORES          # batches per core
NT = M // 128              # m-tiles per batch
KT = CD // 128             # k-tiles of the P contraction
KTC = KT // NCORES         # k-tiles per core (phase A k-sharded)
F32 = mybir.dt.float32
BF16 = mybir.dt.bfloat16
ALU = mybir.AluOpType
AFT = mybir.ActivationFunctionType


def build_nc(reps: int = 1, n_batches: int = BPC, pipeline: bool = True,
             x_bufs: int = 3, x_chunk: int = 8, wopt: bool = False,
             scale_split: bool = False, dual_q: bool = False,
             defer: bool = False, cc_bf16: bool = False,
             pt_bufs: int = 2, fold_bufs: int = 3,
             out_q_scalar: bool = True, at_q_pool: bool = False) -> bass.Bass:
    nc = bass.Bass()
    xs = nc.declare_dram_parameter("xs", [BPC, M, D], BF16, isOutput=False)
    # per-core k-slice of P^T: [KTC*128, D]
    pt = nc.declare_dram_parameter("pt", [KTC * 128, D], BF16, isOutput=False)
    # y for ALL batches over this core's k-slice: [128, KTC, B]
    ys = nc.declare_dram_parameter("ys", [128, KTC, B], BF16, isOutput=False)
    enc = nc.declare_dram_parameter("enc", [BPC, D], F32, isOutput=True)

    with TileContext(nc) as tc:
        with (
            tc.tile_pool(name="const", bufs=1) as const_pool,
            tc.tile_pool(name="ptp", bufs=pt_bufs) as pt_pool,
            tc.tile_pool(name="xp", bufs=x_bufs) as x_pool,
            tc.tile_pool(name="arep", bufs=2) as arep_pool,
            tc.tile_pool(name="small", bufs=1) as small_pool,
            tc.tile_pool(name="tiny", bufs=2) as tiny_pool,
            tc.tile_pool(name="scr", bufs=3) as scr_pool,
            tc.tile_pool(name="fold", bufs=fold_bufs) as fold_pool,
            tc.tile_pool(name="ps", bufs=1, space="PSUM") as psum_pool,
            tc.tile_pool(name="psw", bufs=1, space="PSUM") as psum_w_pool,
            tc.tile_pool(name="pse", bufs=1, space="PSUM") as psum_e_pool,
            tc.tile_pool(name="psb", bufs=1, space="PSUM") as psum_b_pool,
            tc.tile_pool(name="dram", bufs=2, space="DRAM") as dram_pool,
        ):
            ones_sb = const_pool.tile([1, 128], BF16)
            nc.vector.memset(ones_sb[:], 1.0)
            ones_col = const_pool.tile([128, 1], F32)
            nc.vector.memset(ones_col[:], 1.0)
            nshift = const_pool.tile([128, 1], F32)
            nc.vector.memset(nshift[:], -16.0)
            ys_sb = const_pool.tile([128, KTC, B], BF16)
            # ys/a_row ride the Activation hwdge queue so the SP queue's
            # xb transfers are not blocked behind the phase-A dependency
            nc.scalar.dma_start(out=ys_sb[:], in_=ys[:])

            # banded matrices for W = S4 @ p (4-tap sliding-window sum done
            # as matmuls in partition space): S4[j, m] = 1/Q iff j-m in
            # {-1, 0, 1, 2}; corner matrices carry the inter-tile halo. The
            # 1/Q entry folds the final "/Q" divide into the band weights.
            FILL = 1.0 / Q
            s4 = const_pool.tile([128, 128], F32)
            nc.gpsimd.memset(s4[:], 0.0)
            for base in (1, 0, -1, -2):
                nc.gpsimd.affine_select(
                    out=s4[:], in_=s4[:], compare_op=ALU.not_equal, fill=FILL,
                    base=base, pattern=[[-1, 128]], channel_multiplier=1,
                )
            sprev = const_pool.tile([128, 128], F32)
            nc.gpsimd.memset(sprev[:], 0.0)
            nc.gpsimd.affine_select(
                out=sprev[:], in_=sprev[:], compare_op=ALU.not_equal, fill=FILL,
                base=-127, pattern=[[-1, 128]], channel_multiplier=1,
            )
            snext = const_pool.tile([128, 128], F32)
            nc.gpsimd.memset(snext[:], 0.0)
            for base in (126, 127):
                nc.gpsimd.affine_select(
                    out=snext[:], in_=snext[:], compare_op=ALU.not_equal,
                    fill=FILL, base=base, pattern=[[-1, 128]],
                    channel_multiplier=1,
                )
            zrow = const_pool.tile([1, 128], F32)
            nc.vector.memset(zrow[:], 0.0)

            def phase_a_pre():
                # ---- Phase A (k-sharded): each core contracts its 1/8
                # k-slice of P^T against y for ALL batches, then a
                # ReduceScatter sums the partials and hands each core the
                # rows of its own 4 batches. Split into pre (up to the
                # collective trigger) and post (everything that waits on the
                # collective) so phase A for rep i+1 can be issued around
                # rep i's phase B without stalling the in-order engines. ----
                pa0 = psum_pool.tile([B, 512], F32, tag="pa0")
                pa1 = psum_pool.tile([B, 512], F32, tag="pa1")
                ptt = pt_pool.tile([128, KTC, D], BF16, tag="ptt")
                nc.sync.dma_start(
                    out=ptt[:],
                    in_=pt[:, :].rearrange("(u p) d -> p u d", p=128),
                )
                for t in range(KTC):
                    for dh, pa in enumerate((pa0, pa1)):
                        nc.tensor.matmul(
                            pa[:],
                            lhsT=ys_sb[:, t, :],
                            rhs=ptt[:, t, dh * 512:(dh + 1) * 512],
                            start=(t == 0),
                            stop=(t == KTC - 1),
                        )
                ccdt = BF16 if cc_bf16 else F32
                aT32f = small_pool.tile([B, D], ccdt, tag="aT32f")
                nc.scalar.copy(out=aT32f[:, 0:512], in_=pa0[:])
                nc.scalar.copy(out=aT32f[:, 512:1024], in_=pa1[:])
                cc_in = dram_pool.tile([B, D], ccdt, tag="cc_in")
                cc_out = dram_pool.tile([BPC, D], ccdt, tag="cc_out")
                nc.gpsimd.dma_start(out=cc_in[:], in_=aT32f[:])
                nc.gpsimd.collective_compute(
                    "ReduceScatter",
                    mybir.AluOpType.add,
                    replica_groups=[list(range(NCORES))],
                    ins=[cc_in.opt()],
                    outs=[cc_out.opt()],
                )
                return cc_out

            arep_gen = 0

            def phase_a_post(cc_out):
                if cc_bf16:
                    # the scattered rows are already bf16: land them directly
                    aT16 = small_pool.tile([BPC, D], BF16, tag="aT")
                    nc.sync.dma_start(out=aT16[:], in_=cc_out[:])
                else:
                    aT32m = small_pool.tile([BPC, D], F32, tag="aT32m")
                    # on SP this trigger's collective-wait would block the
                    # next rep's ptt/xb prefetch behind it in the queue
                    aeng = nc.gpsimd if at_q_pool else nc.sync
                    aeng.dma_start(out=aT32m[:], in_=cc_out[:])
                    aT16 = small_pool.tile([BPC, D], BF16, tag="aT")
                    nc.scalar.copy(out=aT16[:], in_=aT32m[:])

                # replicate a[b] across all 128 partitions (ones ⊗ a-row on
                # PE; the partition_broadcast ISA op does not codegen on this
                # walrus), then duplicate along the free dim on DVE so the
                # phase-B pair-multiplies see [128, 2, D].
                nonlocal arep_gen
                arep_gen += 1
                a_rep = [
                    arep_pool.tile([128, 2, D], BF16, tag=f"a_rep{b}",
                                   name=f"a_rep{b}_g{arep_gen}")
                    for b in range(BPC)
                ]
                for b in range(BPC):
                    a_row = small_pool.tile([1, D], BF16, tag="a_row")
                    nc.scalar.dma_start(out=a_row[:], in_=aT16[b:b + 1, :])
                    for dh in range(2):
                        pr = psum_b_pool.tile([128, 512], F32, tag="pr")
                        nc.tensor.matmul(
                            pr[:],
                            lhsT=ones_sb[:],
                            rhs=a_row[:, dh * 512:(dh + 1) * 512],
                            start=True,
                            stop=True,
                        )
                        nc.scalar.copy(
                            out=a_rep[b][:, 0, dh * 512:(dh + 1) * 512],
                            in_=pr[:],
                        )
                    nc.vector.tensor_copy(a_rep[b][:, 1, :], a_rep[b][:, 0, :])
                return a_rep

            # Deferred batch tails (1/Z + final scale + output DMA): the
            # scale depends on the enc matmul chain (~14us of PE), and ACT
            # is in-order — emitting it inline stalls ACT between exp(b)
            # and the next batch's accumulates. Deferring it into the next
            # batch (reciprocal at batch start on DVE, scale right after
            # exp on ACT) makes every read hit long-completed data.
            pending_tail = []

            def emit_tail_rz(t):
                rz = tiny_pool.tile([1, 1], F32, tag="rz")
                nc.vector.reciprocal(rz[:], t["z_ps"][:])
                t["rz"] = rz

            def emit_tail_scale(t):
                enc_sb = tiny_pool.tile([1, D], F32, tag="enc_sb")
                nc.scalar.activation(
                    out=enc_sb[:, 0:512], in_=t["pe0"][:], func=AFT.Copy,
                    scale=t["rz"][:],
                )
                nc.scalar.activation(
                    out=enc_sb[:, 512:1024], in_=t["pe1"][:], func=AFT.Copy,
                    scale=t["rz"][:],
                )
                nc.sync.dma_start(out=enc[t["b"], :], in_=enc_sb[0:1, :])

            def phase_b(a_rep):
                for b in range(n_batches):
                    if defer and pending_tail:
                        emit_tail_rz(pending_tail[0])
                    xb = x_pool.tile([128, NT, D], BF16, tag="xb")
                    ch = x_chunk
                    for t in range(NT // ch):
                        eng = (nc.scalar if dual_q and t % 2 else nc.sync)
                        eng.dma_start(
                            out=xb[:, ch * t:ch * t + ch, :],
                            in_=xs[b, t * 128 * ch:(t + 1) * 128 * ch, :]
                            .rearrange("(u p) d -> p u d", p=128),
                        )

                    # logits[m] = x[m, :] . a — DVE multiply over two m-tiles
                    # per op, then tree-folds spread across DVE/GpSimd and a
                    # ScalarE Copy-with-accumulate per tile. (The fused
                    # TENSOR_TENSOR_REDUCE / AFFINE_MUL_REDUCE DVE ops are
                    # rejected by this walrus build: "ISA wrong length".)
                    # Pairs 0..4: DVE fold to 512, GpSimd fold to 256, ACT
                    # accumulates 256. Pairs 5..7: GpSimd fold to 512, ACT
                    # accumulates 512. Balances DVE/ACT/Pool near the per-
                    # batch DMA floor.
                    logits_sb = tiny_pool.tile([128, NT], F32, tag="logits")
                    for j in range(NT // 2):
                        scr2 = scr_pool.tile([128, 2, D], BF16, tag="scr2")
                        nc.vector.tensor_mul(
                            scr2[:], xb[:, 2 * j:2 * j + 2, :], a_rep[b][:]
                        )
                        if j < 6:
                            # DVE fold to 512, GpSimd fold to 256, ACT-256
                            half = fold_pool.tile([128, 2, 512], BF16,
                                                  tag="half")
                            nc.vector.tensor_add(
                                half[:], scr2[:, :, 0:512],
                                scr2[:, :, 512:1024]
                            )
                            red = fold_pool.tile([128, 2, 256], BF16, tag="q")
                            nc.gpsimd.tensor_add(
                                red[:], half[:, :, 0:256], half[:, :, 256:512]
                            )
                        else:
                            # GpSimd fold to 512, ACT-512
                            red = fold_pool.tile([128, 2, 512], BF16,
                                                 tag="half")
                            nc.gpsimd.tensor_add(
                                red[:], scr2[:, :, 0:512], scr2[:, :, 512:1024]
                            )
                        for u in range(2):
                            t = 2 * j + u
                            nc.scalar.activation(
                                out=red[:, u, :],
                                in_=red[:, u, :],
                                func=AFT.Copy,
                                accum_out=logits_sb[:, t:t + 1],
                            )

                    # softmax without the row gather: a FIXED shift replaces
                    # the max (it cancels exactly in enc = sum(W x)/(2Z)),
                    # so exp runs in [128, NT] partition space on ACT.
                    p_sb = tiny_pool.tile([128, NT], F32, tag="p_sb")
                    zcol = tiny_pool.tile([128, 1], F32, tag="zcol")
                    nc.scalar.activation(
                        out=p_sb[:],
                        in_=logits_sb[:],
                        func=AFT.Exp,
                        bias=nshift[:],
                        scale=1.0,
                        accum_out=zcol[:],
                    )

                    # Deferred tail of the previous batch: its enc PSUM and
                    # Z are long done, so these ACT ops issue stall-free.
                    if defer and pending_tail:
                        emit_tail_scale(pending_tail.pop(0))

                    # Z = sum over partitions of zcol (ones-column matmul),
                    # then 1/Z on DVE straight from PSUM (the /Q is folded
                    # into the band fills).
                    z_ps = psum_w_pool.tile([1, 1], F32, tag="z_ps")
                    nc.tensor.matmul(z_ps[:], lhsT=zcol[:], rhs=ones_col[:],
                                     start=True, stop=True)
                    if not defer:
                        rz = tiny_pool.tile([1, 1], F32, tag="rz")
                        nc.vector.reciprocal(rz[:], z_ps[:])

                    # W[m] = p[m-1]+p[m]+p[m+1]+p[m+2] via banded matmuls;
                    # the inter-tile halo terms act on p shifted by one tile
                    # column (zero-padded at the ends).
                    w_ps = psum_w_pool.tile([128, NT], F32, tag="w_ps")
                    if wopt:
                        # column-shifted PSUM subranges instead of shifted
                        # copies of p; a zero matmul closes the accumulation
                        # group over the full range (stop is sim-only).
                        nc.tensor.matmul(w_ps[:], lhsT=s4[:], rhs=p_sb[:],
                                         start=True, stop=False)
                        nc.tensor.matmul(w_ps[:, 1:NT], lhsT=sprev[:],
                                         rhs=p_sb[:, 0:NT - 1],
                                         start=False, stop=False)
                        nc.tensor.matmul(w_ps[:, 0:NT - 1], lhsT=snext[:],
                                         rhs=p_sb[:, 1:NT],
                                         start=False, stop=False)
                        nc.tensor.matmul(w_ps[:], lhsT=zrow[:],
                                         rhs=p_sb[0:1, :],
                                         start=False, stop=True)
                    else:
                        p_prev = tiny_pool.tile([128, NT], F32, tag="p_prev")
                        nc.gpsimd.memset(p_prev[:, 0:1], 0.0)
                        nc.gpsimd.tensor_copy(p_prev[:, 1:NT],
                                              p_sb[:, 0:NT - 1])
                        p_next = tiny_pool.tile([128, NT], F32, tag="p_next")
                        nc.gpsimd.memset(p_next[:, NT - 1:NT], 0.0)
                        nc.gpsimd.tensor_copy(p_next[:, 0:NT - 1],
                                              p_sb[:, 1:NT])
                        nc.tensor.matmul(w_ps[:], lhsT=s4[:], rhs=p_sb[:],
                                         start=True, stop=False)
                        nc.tensor.matmul(w_ps[:], lhsT=sprev[:], rhs=p_prev[:],
                                         start=False, stop=False)
                        nc.tensor.matmul(w_ps[:], lhsT=snext[:], rhs=p_next[:],
                                         start=False, stop=True)
                    w_pm = tiny_pool.tile([128, NT], BF16, tag="w_pm")
                    if defer:
                        # adjacent to exp on ACT: no cross-engine stall
                        nc.scalar.copy(out=w_pm[:], in_=w_ps[:])
                    else:
                        nc.vector.tensor_copy(w_pm[:], w_ps[:])

                    # enc_un[d] = sum_m W[m] x[m, d]   (PE, W as 1-col
                    # weights, 1024-wide rhs into a 2-bank PSUM tile).
                    # The last tile contracts over 127 partitions only: this
                    # drops m = M-1, enforcing W[M-1] = 0 (exclusive slice end
                    # in the reference) without touching w_pm.
                    pe0 = psum_e_pool.tile([1, 512], F32, tag="pe0")
                    pe1 = psum_e_pool.tile([1, 512], F32, tag="pe1")
                    for t in range(NT):
                        pp = 127 if t == NT - 1 else 128
                        for dh, pe in enumerate((pe0, pe1)):
                            nc.tensor.matmul(
                                pe[:],
                                lhsT=w_pm[0:pp, t:t + 1],
                                rhs=xb[0:pp, t, dh * 512:(dh + 1) * 512],
                                start=(t == 0),
                                stop=(t == NT - 1),
                            )

                    # enc[b] = enc_un / Z  (the /Q went into the band fills)
                    if defer:
                        pending_tail.append(
                            {"pe0": pe0, "pe1": pe1, "z_ps": z_ps, "b": b}
                        )
                        continue
                    enc_sb = small_pool.tile([1, D], F32, tag="enc_sb")
                    nc.scalar.activation(
                        out=enc_sb[:, 0:512], in_=pe0[:], func=AFT.Copy,
                        scale=rz[:],
                    )
                    if scale_split:
                        nc.vector.tensor_scalar_mul(
                            enc_sb[:, 512:1024], pe1[:], rz[:]
                        )
                    else:
                        nc.scalar.activation(
                            out=enc_sb[:, 512:1024], in_=pe1[:], func=AFT.Copy,
                            scale=rz[:],
                        )
                    oeng = nc.scalar if out_q_scalar else nc.sync
                    oeng.dma_start(out=enc[b, :], in_=enc_sb[0:1, :])

            # Python-unrolled reps (collective_compute does not codegen
            # inside a For_i hardware loop on this walrus build), software-
            # pipelined: rep i's phase B runs while rep i+1's phase A
            # (P-slice matmul + ReduceScatter) is in flight; the collective-
            # dependent tail of phase A is emitted after phase B so no
            # in-order engine stalls waiting on the collective.
            if pipeline:
                a_rep = phase_a_post(phase_a_pre())
                for i in range(reps):
                    cc_next = phase_a_pre() if i + 1 < reps else None
                    phase_b(a_rep)
                    if cc_next is not None:
                        a_rep = phase_a_post(cc_next)
            else:
                for i in range(reps):
                    phase_b(phase_a_post(phase_a_pre()))
            while pending_tail:
                t = pending_tail.pop(0)
                emit_tail_rz(t)
                emit_tail_scale(t)

    return nc


def _shard_inputs(embeds_x, embeds_y, P):
    """Build the 8 per-core input maps (host-side resharding + bf16 cast).

    x is data-parallel over batch (4 per core). Phase A is k-sharded: core c
    gets P^T rows [c*640, (c+1)*640) and y[:, same k-slice] for ALL batches;
    the kernel ReduceScatters the partial a's.
    """
    bf16 = ml_dtypes.bfloat16
    x = np.asarray(embeds_x, dtype=np.float32).astype(bf16)
    y = np.asarray(embeds_y, dtype=np.float32)[:, :, 0].astype(bf16)  # [B, CD]
    pt = np.ascontiguousarray(
        np.asarray(P, dtype=np.float32).T.astype(bf16))      # [CD, D]
    kc = KTC * 128
    in_maps = []
    for c in range(NCORES):
        sl = slice(c * BPC, (c + 1) * BPC)
        ys_c = np.ascontiguousarray(
            y[:, c * kc:(c + 1) * kc].reshape(B, KTC, 128).transpose(2, 1, 0)
        )  # [128, KTC, B]
        in_maps.append({
            "xs": np.ascontiguousarray(x[sl]),
            "pt": np.ascontiguousarray(pt[c * kc:(c + 1) * kc]),
            "ys": ys_c,
        })
    return in_maps


def kernel(embeds_x, embeds_y, P, M):
    assert int(M) == 2048
    nc = build_nc(reps=1)
    split_sync_waits(nc)  # HW-compile only; CoreSim rejects injected NoOps
    in_maps = _shard_inputs(embeds_x, embeds_y, P)
    res = run_bass_kernel_spmd(nc, in_maps, list(range(NCORES)))
    out = np.concatenate([res.results[c]["enc"] for c in range(NCORES)], axis=0)
    return out.astype(np.float32)


# revision 69
# speedup vs baseline: 1.1634x; 1.0535x over previous
"""Trainium2 Bass kernel for the attention-based encoder.

Computation (per batch b):
    a      = P @ y[b]                                  # [D]
    logits = x[b] @ a                                  # [M]
    p_un   = exp(logits - 16); Z = sum(p_un)
    W[t]   = p_un[t-1] + p_un[t] + p_un[t+1] + p_un[t+2]  (zero-padded), W[M-1] = 0
    enc[b] = (W @ x[b]) / (Q * Z)                      # [D]

which is algebraically identical to the reference (cumsum sliding window +
bilinear softmax attention), with the smoothing window folded onto the softmax
weights instead of the embeddings so x[b] is only needed in natural layout.

Sharding: data-parallel over batch, 4 batches per core on 8 cores. P is
replicated (passed pre-transposed so the contraction dim lands on SBUF
partitions without on-chip transposes).

Precision: x, P and y are cast to bf16 on the host (halves HBM traffic, the
dominant cost, and runs the PE/DVE paths at 2x fp32 rate). All reductions
(logit accumulate, softmax Z, PSUM matmul accumulate) stay fp32. Measured
end-to-end rel err vs the fp32 reference is ~5e-3.
"""

import numpy as np
import ml_dtypes

import concourse.bass as bass
import concourse.mybir as mybir
from concourse.tile import TileContext
from concourse.bass_utils import run_bass_kernel_spmd

# ---------------------------------------------------------------------------
# This container's walrus supports only ONE sync wait per instruction ("Too
# many sync wait commands" at codegen otherwise), while Tile freely attaches
# several.  Post-pass: hoist excess waits onto injected same-engine NoOps
# placed immediately before the over-subscribed instruction.
# ---------------------------------------------------------------------------

_MAX_WAITS = 1


def split_sync_waits(nc: bass.Bass) -> None:
    uid = 0
    for fn in nc.m.functions:
        for blk in fn.blocks:
            new_insts = []
            for inst in blk.instructions:
                si = inst.sync_info
                waits = list(si.on_wait) if si and si.on_wait else []
                if len(waits) > _MAX_WAITS:
                    for w in waits[:-_MAX_WAITS]:
                        uid += 1
                        ev = mybir.InstEventSemaphore(
                            name=f"{inst.name}_hw{uid}",
                            opcode="EventSemaphore",
                            ins=[],
                            outs=[],
                            sync_info=mybir.SyncInfo(on_wait=[w], on_update=[]),
                        )
                        ev.engine = inst.engine
                        new_insts.append(ev)
                    si.on_wait = waits[-_MAX_WAITS:]
                new_insts.append(inst)
            blk.instructions[:] = new_insts

# ---------------------------------------------------------------------------

B, M, D, CD = 32, 2048, 1024, 5120
Q = 2
NCORES = 8
BPC = B // NCORES          # batches per core
NT = M // 128              # m-tiles per batch
KT = CD // 128             # k-tiles of the P contraction
KTC = KT // NCORES         # k-tiles per core (phase A k-sharded)
F32 = mybir.dt.float32
BF16 = mybir.dt.bfloat16
ALU = mybir.AluOpType
AFT = mybir.ActivationFunctionType


def build_nc(reps: int = 1, n_batches: int = BPC, pipeline: bool = True,
             x_bufs: int = 3, x_chunk: int = 8, wopt: bool = False,
             scale_split: bool = False, dual_q: bool = False,
             defer: bool = False, cc_bf16: bool = False,
             pt_bufs: int = 2, fold_bufs: int = 3,
             out_q_scalar: bool = True, at_q_pool: bool = False,
             scr_bufs: int = 3) -> bass.Bass:
    nc = bass.Bass()
    xs = nc.declare_dram_parameter("xs", [BPC, M, D], BF16, isOutput=False)
    # per-core k-slice of P^T: [KTC*128, D]
    pt = nc.declare_dram_parameter("pt", [KTC * 128, D], BF16, isOutput=False)
    # y for ALL batches over this core's k-slice: [128, KTC, B]
    ys = nc.declare_dram_parameter("ys", [128, KTC, B], BF16, isOutput=False)
    enc = nc.declare_dram_parameter("enc", [BPC, D], F32, isOutput=True)

    with TileContext(nc) as tc:
        with (
            tc.tile_pool(name="const", bufs=1) as const_pool,
            tc.tile_pool(name="ptp", bufs=pt_bufs) as pt_pool,
            tc.tile_pool(name="xp", bufs=x_bufs) as x_pool,
            tc.tile_pool(name="arep", bufs=2) as arep_pool,
            tc.tile_pool(name="small", bufs=1) as small_pool,
            tc.tile_pool(name="tiny", bufs=2) as tiny_pool,
            tc.tile_pool(name="scr", bufs=scr_bufs) as scr_pool,
            tc.tile_pool(name="fold", bufs=fold_bufs) as fold_pool,
            tc.tile_pool(name="ps", bufs=1, space="PSUM") as psum_pool,
            tc.tile_pool(name="psw", bufs=1, space="PSUM") as psum_w_pool,
            tc.tile_pool(name="pse", bufs=1, space="PSUM") as psum_e_pool,
            tc.tile_pool(name="psb", bufs=1, space="PSUM") as psum_b_pool,
            tc.tile_pool(name="dram", bufs=2, space="DRAM") as dram_pool,
        ):
            ones_sb = const_pool.tile([1, 128], BF16)
            nc.vector.memset(ones_sb[:], 1.0)
            ones_col = const_pool.tile([128, 1], F32)
            nc.vector.memset(ones_col[:], 1.0)
            nshift = const_pool.tile([128, 1], F32)
            nc.vector.memset(nshift[:], -16.0)
            ys_sb = const_pool.tile([128, KTC, B], BF16)
            # ys/a_row ride the Activation hwdge queue so the SP queue's
            # xb transfers are not blocked behind the phase-A dependency
            nc.scalar.dma_start(out=ys_sb[:], in_=ys[:])

            # banded matrices for W = S4 @ p (4-tap sliding-window sum done
            # as matmuls in partition space): S4[j, m] = 1/Q iff j-m in
            # {-1, 0, 1, 2}; corner matrices carry the inter-tile halo. The
            # 1/Q entry folds the final "/Q" divide into the band weights.
            FILL = 1.0 / Q
            s4 = const_pool.tile([128, 128], F32)
            nc.gpsimd.memset(s4[:], 0.0)
            for base in (1, 0, -1, -2):
                nc.gpsimd.affine_select(
                    out=s4[:], in_=s4[:], compare_op=ALU.not_equal, fill=FILL,
                    base=base, pattern=[[-1, 128]], channel_multiplier=1,
                )
            sprev = const_pool.tile([128, 128], F32)
            nc.gpsimd.memset(sprev[:], 0.0)
            nc.gpsimd.affine_select(
                out=sprev[:], in_=sprev[:], compare_op=ALU.not_equal, fill=FILL,
                base=-127, pattern=[[-1, 128]], channel_multiplier=1,
            )
            snext = const_pool.tile([128, 128], F32)
            nc.gpsimd.memset(snext[:], 0.0)
            for base in (126, 127):
                nc.gpsimd.affine_select(
                    out=snext[:], in_=snext[:], compare_op=ALU.not_equal,
                    fill=FILL, base=base, pattern=[[-1, 128]],
                    channel_multiplier=1,
                )
            zrow = const_pool.tile([1, 128], F32)
            nc.vector.memset(zrow[:], 0.0)

            def phase_a_pre():
                # ---- Phase A (k-sharded): each core contracts its 1/8
                # k-slice of P^T against y for ALL batches, then a
                # ReduceScatter sums the partials and hands each core the
                # rows of its own 4 batches. Split into pre (up to the
                # collective trigger) and post (everything that waits on the
                # collective) so phase A for rep i+1 can be issued around
                # rep i's phase B without stalling the in-order engines. ----
                pa0 = psum_pool.tile([B, 512], F32, tag="pa0")
                pa1 = psum_pool.tile([B, 512], F32, tag="pa1")
                ptt = pt_pool.tile([128, KTC, D], BF16, tag="ptt")
                nc.sync.dma_start(
                    out=ptt[:],
                    in_=pt[:, :].rearrange("(u p) d -> p u d", p=128),
                )
                for t in range(KTC):
                    for dh, pa in enumerate((pa0, pa1)):
                        nc.tensor.matmul(
                            pa[:],
                            lhsT=ys_sb[:, t, :],
                            rhs=ptt[:, t, dh * 512:(dh + 1) * 512],
                            start=(t == 0),
                            stop=(t == KTC - 1),
                        )
                ccdt = BF16 if cc_bf16 else F32
                aT32f = small_pool.tile([B, D], ccdt, tag="aT32f")
                nc.scalar.copy(out=aT32f[:, 0:512], in_=pa0[:])
                nc.scalar.copy(out=aT32f[:, 512:1024], in_=pa1[:])
                cc_in = dram_pool.tile([B, D], ccdt, tag="cc_in")
                cc_out = dram_pool.tile([BPC, D], ccdt, tag="cc_out")
                nc.gpsimd.dma_start(out=cc_in[:], in_=aT32f[:])
                nc.gpsimd.collective_compute(
                    "ReduceScatter",
                    mybir.AluOpType.add,
                    replica_groups=[list(range(NCORES))],
                    ins=[cc_in.opt()],
                    outs=[cc_out.opt()],
                )
                return cc_out

            arep_gen = 0

            def phase_a_post(cc_out):
                if cc_bf16:
                    # the scattered rows are already bf16: land them directly
                    aT16 = small_pool.tile([BPC, D], BF16, tag="aT")
                    nc.sync.dma_start(out=aT16[:], in_=cc_out[:])
                else:
                    aT32m = small_pool.tile([BPC, D], F32, tag="aT32m")
                    # on SP this trigger's collective-wait would block the
                    # next rep's ptt/xb prefetch behind it in the queue
                    aeng = nc.gpsimd if at_q_pool else nc.sync
                    aeng.dma_start(out=aT32m[:], in_=cc_out[:])
                    aT16 = small_pool.tile([BPC, D], BF16, tag="aT")
                    nc.scalar.copy(out=aT16[:], in_=aT32m[:])

                # replicate a[b] across all 128 partitions (ones ⊗ a-row on
                # PE; the partition_broadcast ISA op does not codegen on this
                # walrus), then duplicate along the free dim on DVE so the
                # phase-B pair-multiplies see [128, 2, D].
                nonlocal arep_gen
                arep_gen += 1
                a_rep = [
                    arep_pool.tile([128, 2, D], BF16, tag=f"a_rep{b}",
                                   name=f"a_rep{b}_g{arep_gen}")
                    for b in range(BPC)
                ]
                for b in range(BPC):
                    a_row = small_pool.tile([1, D], BF16, tag="a_row")
                    nc.scalar.dma_start(out=a_row[:], in_=aT16[b:b + 1, :])
                    for dh in range(2):
                        pr = psum_b_pool.tile([128, 512], F32, tag="pr")
                        nc.tensor.matmul(
                            pr[:],
                            lhsT=ones_sb[:],
                            rhs=a_row[:, dh * 512:(dh + 1) * 512],
                            start=True,
                            stop=True,
                        )
                        nc.scalar.copy(
                            out=a_rep[b][:, 0, dh * 512:(dh + 1) * 512],
                            in_=pr[:],
                        )
                    nc.vector.tensor_copy(a_rep[b][:, 1, :], a_rep[b][:, 0, :])
                return a_rep

            # Deferred batch tails (1/Z + final scale + output DMA): the
            # scale depends on the enc matmul chain (~14us of PE), and ACT
            # is in-order — emitting it inline stalls ACT between exp(b)
            # and the next batch's accumulates. Deferring it into the next
            # batch (reciprocal at batch start on DVE, scale right after
            # exp on ACT) makes every read hit long-completed data.
            pending_tail = []

            def emit_tail_rz(t):
                rz = tiny_pool.tile([1, 1], F32, tag="rz")
                nc.vector.reciprocal(rz[:], t["z_ps"][:])
                t["rz"] = rz

            def emit_tail_scale(t):
                enc_sb = tiny_pool.tile([1, D], F32, tag="enc_sb")
                nc.scalar.activation(
                    out=enc_sb[:, 0:512], in_=t["pe0"][:], func=AFT.Copy,
                    scale=t["rz"][:],
                )
                nc.scalar.activation(
                    out=enc_sb[:, 512:1024], in_=t["pe1"][:], func=AFT.Copy,
                    scale=t["rz"][:],
                )
                nc.sync.dma_start(out=enc[t["b"], :], in_=enc_sb[0:1, :])

            def phase_b(a_rep):
                for b in range(n_batches):
                    if defer and pending_tail:
                        emit_tail_rz(pending_tail[0])
                    xb = x_pool.tile([128, NT, D], BF16, tag="xb")
                    ch = x_chunk
                    for t in range(NT // ch):
                        eng = (nc.scalar if dual_q and t % 2 else nc.sync)
                        eng.dma_start(
                            out=xb[:, ch * t:ch * t + ch, :],
                            in_=xs[b, t * 128 * ch:(t + 1) * 128 * ch, :]
                            .rearrange("(u p) d -> p u d", p=128),
                        )

                    # logits[m] = x[m, :] . a — DVE multiply over two m-tiles
                    # per op, then tree-folds spread across DVE/GpSimd and a
                    # ScalarE Copy-with-accumulate per tile. (The fused
                    # TENSOR_TENSOR_REDUCE / AFFINE_MUL_REDUCE DVE ops are
                    # rejected by this walrus build: "ISA wrong length".)
                    # Pairs 0..4: DVE fold to 512, GpSimd fold to 256, ACT
                    # accumulates 256. Pairs 5..7: GpSimd fold to 512, ACT
                    # accumulates 512. Balances DVE/ACT/Pool near the per-
                    # batch DMA floor.
                    logits_sb = tiny_pool.tile([128, NT], F32, tag="logits")
                    for j in range(NT // 2):
                        scr2 = scr_pool.tile([128, 2, D], BF16, tag="scr2")
                        nc.vector.tensor_mul(
                            scr2[:], xb[:, 2 * j:2 * j + 2, :], a_rep[b][:]
                        )
                        if j < 6:
                            # DVE fold to 512, GpSimd fold to 256, ACT-256
                            half = fold_pool.tile([128, 2, 512], BF16,
                                                  tag="half")
                            nc.vector.tensor_add(
                                half[:], scr2[:, :, 0:512],
                                scr2[:, :, 512:1024]
                            )
                            red = fold_pool.tile([128, 2, 256], BF16, tag="q")
                            nc.gpsimd.tensor_add(
                                red[:], half[:, :, 0:256], half[:, :, 256:512]
                            )
                        else:
                            # GpSimd fold to 512, ACT-512
                            red = fold_pool.tile([128, 2, 512], BF16,
                                                 tag="half")
                            nc.gpsimd.tensor_add(
                                red[:], scr2[:, :, 0:512], scr2[:, :, 512:1024]
                            )
                        for u in range(2):
                            t = 2 * j + u
                            nc.scalar.activation(
                                out=red[:, u, :],
                                in_=red[:, u, :],
                                func=AFT.Copy,
                                accum_out=logits_sb[:, t:t + 1],
                            )

                    # softmax without the row gather: a FIXED shift replaces
                    # the max (it cancels exactly in enc = sum(W x)/(2Z)),
                    # so exp runs in [128, NT] partition space on ACT.
                    p_sb = tiny_pool.tile([128, NT], F32, tag="p_sb")
                    zcol = tiny_pool.tile([128, 1], F32, tag="zcol")
                    nc.scalar.activation(
                        out=p_sb[:],
                        in_=logits_sb[:],
                        func=AFT.Exp,
                        bias=nshift[:],
                        scale=1.0,
                        accum_out=zcol[:],
                    )

                    # Deferred tail of the previous batch: its enc PSUM and
                    # Z are long done, so these ACT ops issue stall-free.
                    if defer and pending_tail:
                        emit_tail_scale(pending_tail.pop(0))

                    # Z = sum over partitions of zcol (ones-column matmul),
                    # then 1/Z on DVE straight from PSUM (the /Q is folded
                    # into the band fills).
                    z_ps = psum_w_pool.tile([1, 1], F32, tag="z_ps")
                    nc.tensor.matmul(z_ps[:], lhsT=zcol[:], rhs=ones_col[:],
                                     start=True, stop=True)
                    if not defer:
                        rz = tiny_pool.tile([1, 1], F32, tag="rz")
                        nc.vector.reciprocal(rz[:], z_ps[:])

                    # W[m] = p[m-1]+p[m]+p[m+1]+p[m+2] via banded matmuls;
                    # the inter-tile halo terms act on p shifted by one tile
                    # column (zero-padded at the ends).
                    w_ps = psum_w_pool.tile([128, NT], F32, tag="w_ps")
                    if wopt:
                        # column-shifted PSUM subranges instead of shifted
                        # copies of p; a zero matmul closes the accumulation
                        # group over the full range (stop is sim-only).
                        nc.tensor.matmul(w_ps[:], lhsT=s4[:], rhs=p_sb[:],
                                         start=True, stop=False)
                        nc.tensor.matmul(w_ps[:, 1:NT], lhsT=sprev[:],
                                         rhs=p_sb[:, 0:NT - 1],
                                         start=False, stop=False)
                        nc.tensor.matmul(w_ps[:, 0:NT - 1], lhsT=snext[:],
                                         rhs=p_sb[:, 1:NT],
                                         start=False, stop=False)
                        nc.tensor.matmul(w_ps[:], lhsT=zrow[:],
                                         rhs=p_sb[0:1, :],
                                         start=False, stop=True)
                    else:
                        p_prev = tiny_pool.tile([128, NT], F32, tag="p_prev")
                        nc.gpsimd.memset(p_prev[:, 0:1], 0.0)
                        nc.gpsimd.tensor_copy(p_prev[:, 1:NT],
                                              p_sb[:, 0:NT - 1])
                        p_next = tiny_pool.tile([128, NT], F32, tag="p_next")
                        nc.gpsimd.memset(p_next[:, NT - 1:NT], 0.0)
                        nc.gpsimd.tensor_copy(p_next[:, 0:NT - 1],
                                              p_sb[:, 1:NT])
                        nc.tensor.matmul(w_ps[:], lhsT=s4[:], rhs=p_sb[:],
                                         start=True, stop=False)
                        nc.tensor.matmul(w_ps[:], lhsT=sprev[:], rhs=p_prev[:],
                                         start=False, stop=False)
                        nc.tensor.matmul(w_ps[:], lhsT=snext[:], rhs=p_next[:],
                                         start=False, stop=True)
                    w_pm = tiny_pool.tile([128, NT], BF16, tag="w_pm")
                    if defer:
                        # adjacent to exp on ACT: no cross-engine stall
                        nc.scalar.copy(out=w_pm[:], in_=w_ps[:])
                    else:
                        nc.vector.tensor_copy(w_pm[:], w_ps[:])

                    # enc_un[d] = sum_m W[m] x[m, d]   (PE, W as 1-col
                    # weights, 1024-wide rhs into a 2-bank PSUM tile).
                    # The last tile contracts over 127 partitions only: this
                    # drops m = M-1, enforcing W[M-1] = 0 (exclusive slice end
                    # in the reference) without touching w_pm.
                    pe0 = psum_e_pool.tile([1, 512], F32, tag="pe0")
                    pe1 = psum_e_pool.tile([1, 512], F32, tag="pe1")
                    for t in range(NT):
                        pp = 127 if t == NT - 1 else 128
                        for dh, pe in enumerate((pe0, pe1)):
                            nc.tensor.matmul(
                                pe[:],
                                lhsT=w_pm[0:pp, t:t + 1],
                                rhs=xb[0:pp, t, dh * 512:(dh + 1) * 512],
                                start=(t == 0),
                                stop=(t == NT - 1),
                            )

                    # enc[b] = enc_un / Z  (the /Q went into the band fills)
                    if defer:
                        pending_tail.append(
                            {"pe0": pe0, "pe1": pe1, "z_ps": z_ps, "b": b}
                        )
                        continue
                    enc_sb = small_pool.tile([1, D], F32, tag="enc_sb")
                    nc.scalar.activation(
                        out=enc_sb[:, 0:512], in_=pe0[:], func=AFT.Copy,
                        scale=rz[:],
                    )
                    if scale_split:
                        nc.vector.tensor_scalar_mul(
                            enc_sb[:, 512:1024], pe1[:], rz[:]
                        )
                    else:
                        nc.scalar.activation(
                            out=enc_sb[:, 512:1024], in_=pe1[:], func=AFT.Copy,
                            scale=rz[:],
                        )
                    oeng = nc.scalar if out_q_scalar else nc.sync
                    oeng.dma_start(out=enc[b, :], in_=enc_sb[0:1, :])

            # Python-unrolled reps (collective_compute does not codegen
            # inside a For_i hardware loop on this walrus build), software-
            # pipelined: rep i's phase B runs while rep i+1's phase A
            # (P-slice matmul + ReduceScatter) is in flight; the collective-
            # dependent tail of phase A is emitted after phase B so no
            # in-order engine stalls waiting on the collective.
            if pipeline:
                a_rep = phase_a_post(phase_a_pre())
                for i in range(reps):
                    cc_next = phase_a_pre() if i + 1 < reps else None
                    phase_b(a_rep)
                    if cc_next is not None:
                        a_rep = phase_a_post(cc_next)
            else:
                for i in range(reps):
                    phase_b(phase_a_post(phase_a_pre()))
            while pending_tail:
                t = pending_tail.pop(0)
                emit_tail_rz(t)
                emit_tail_scale(t)

    return nc


def _shard_inputs(embeds_x, embeds_y, P):
    """Build the 8 per-core input maps (host-side resharding + bf16 cast).

    x is data-parallel over batch (4 per core). Phase A is k-sharded: core c
    gets P^T rows [c*640, (c+1)*640) and y[:, same k-slice] for ALL batches;
    the kernel ReduceScatters the partial a's.
    """
    bf16 = ml_dtypes.bfloat16
    x = np.asarray(embeds_x, dtype=np.float32).astype(bf16)
    y = np.asarray(embeds_y, dtype=np.float32)[:, :, 0].astype(bf16)  # [B, CD]
    pt = np.ascontiguousarray(
        np.asarray(P, dtype=np.float32).T.astype(bf16))      # [CD, D]
    kc = KTC * 128
    in_maps = []
    for c in range(NCORES):
        sl = slice(c * BPC, (c + 1) * BPC)
        ys_c = np.ascontiguousarray(
            y[:, c * kc:(c + 1) * kc].reshape(B, KTC, 128).transpose(2, 1, 0)
        )  # [128, KTC, B]
        in_maps.append({
            "xs": np.ascontiguousarray(x[sl]),
            "pt": np.ascontiguousarray(pt[c * kc:(c + 1) * kc]),
            "ys": ys_c,
        })
    return in_maps


def kernel(embeds_x, embeds_y, P, M):
    assert int(M) == 2048
    nc = build_nc(reps=1)
    split_sync_waits(nc)  # HW-compile only; CoreSim rejects injected NoOps
    in_maps = _shard_inputs(embeds_x, embeds_y, P)
    res = run_bass_kernel_spmd(nc, in_maps, list(range(NCORES)))
    out = np.concatenate([res.results[c]["enc"] for c in range(NCORES)], axis=0)
    return out.astype(np.float32)
